# revision 11
# baseline (speedup 1.0000x reference)
"""Bass/Trainium2 kernel for a 2-layer GAT (nn_GAT_59115929862612).

Strategy (8 NeuronCores, SPMD single NEFF):
- Edge parallelism sharded by *src-node ownership*: core c owns global nodes
  [c*6250, (c+1)*6250) and processes every edge whose src lies in its range.
  Since softmax denominators and the aggregation are segment-sums over src,
  each core produces complete output rows for its owned nodes - no
  cross-core reduction of partial sums is needed.
- Node-id rotation: each core's inputs are rotated so LOCAL node 0 is the
  first owned node. All per-core addressing is then static (same NEFF on
  all 8 cores), with per-core index streams supplied as input data.
- Layer-1 feature transform (h = x @ W, plus attention projections a_src,
  a_dst folded in as extra output columns) is computed replicated on every
  core into a per-core DRAM table; per-edge gathers use dma_gather.
- Segment-sum via one-hot matmul: for each 128-node window, gathered edge
  tiles are weighted by exp(leaky_relu(e)) and accumulated into PSUM with a
  one-hot selection matrix S (S[e, src_rel] = 1) as the stationary operand.
  Appending the exp values as extra rhs columns yields the softmax
  denominators in the same matmuls; the divide happens per output row.
- Between layers a single AllGather shares the layer-2 transformed node
  table (g2 = elu(h') @ W2aug) across cores.

Numerics note: the reference subtracts a global max inside exp for softmax
stability. Attention weights are shift-invariant per src row and the logit
range here is tiny (|e| < ~25), so unshifted exp is exact-safe in f32.
"""

import math
import os

import numpy as np

N_NODES = 50000
N_EDGES = 800000
N_CORES = 8
NPC = N_NODES // N_CORES          # nodes per core
HALF = N_NODES // 2               # dma_gather idx is int16; split tables at 25000
NFEAT = 128
ALPHA = 0.2

# layer 1: 4 heads x 64; layer 2: 2 heads x 121
H1, D1 = 4, 64
H2, D2 = 2, 121
F1 = H1 * D1                      # 256
F2 = H2 * D2                      # 242
AB1 = 2 * H1                      # a-block cols (a_src | a_dst)
AB2 = 2 * H2
R1 = AB1 + F1                     # 264 rhs/psum cols
R2 = AB2 + F2                     # 246
ROW1 = 320                        # table1 row f32 (1280B, mult of 256B)
ROW2 = 256                        # table2 row f32 (1024B)
P = 128


def _cfg_full():
    return dict(
        n=N_NODES, npc=NPC, half=HALF, nwin=math.ceil(NPC / P),
    )


# ---------------------------------------------------------------- host prep

def _prep_weights(W1, a1, b1, W2, a2, b2):
    """W1aug [128, R1] col order [Ãsrc1 | Ãdst1 | W1flat];
    W2aug [2, 128, R2]; b1 row bcast; mean-b2 bcast."""
    W1 = np.asarray(W1, np.float32)
    a1 = np.asarray(a1, np.float32)
    W2 = np.asarray(W2, np.float32)
    a2 = np.asarray(a2, np.float32)
    w1aug = np.zeros((NFEAT, R1), np.float32)
    for h in range(H1):
        w1aug[:, h] = W1[h] @ a1[h, :D1, 0]
        w1aug[:, H1 + h] = W1[h] @ a1[h, D1:, 0]
        w1aug[:, AB1 + h * D1:AB1 + (h + 1) * D1] = W1[h]
    w2aug = np.zeros((F1, R2), np.float32)
    for h in range(H2):
        w2aug[:, h] = W2[h] @ a2[h, :D2, 0]
        w2aug[:, H2 + h] = W2[h] @ a2[h, D2:, 0]
        w2aug[:, AB2 + h * D2:AB2 + (h + 1) * D2] = W2[h]
    b1b = np.tile(np.asarray(b1, np.float32).reshape(1, F1), (P, 1)).copy()
    b2m = np.tile(0.5 * (np.asarray(b2, np.float32)[0] + np.asarray(b2, np.float32)[1]).reshape(1, D2), (P, 1)).copy()
    return w1aug, np.stack([w2aug[:NFEAT], w2aug[NFEAT:]]), b1b, b2m


def _pack_idx(flat):
    """[num] int -> [128, num//16] int16 wrap layout (slot i at [i%16, i//16],
    replicated across the 8 gpsimd partition groups)."""
    num = flat.shape[0]
    a = flat.reshape(num // 16, 16).T.astype(np.int16)   # [16, num//16]
    return np.tile(a, (8, 1))


def _pack_srel(flat):
    """[T*128] -> [128, T] f32 (slot i at [i%128, i//128])."""
    t = flat.shape[0] // P
    return flat.reshape(t, P).T.astype(np.float32).copy()


def _prep_edges(edge_list, cfg):
    """Per-core, per-layer padded edge streams.

    Returns (streams, T_LO, T_HI): streams[c] = dict with ilo1/ihi1/isrc1/
    srel1/ilo2/... arrays of shape [nwin, 128, *].
    """
    n, npc, half, nwin = cfg["n"], cfg["npc"], cfg["half"], cfg["nwin"]
    src = np.asarray(edge_list[0], np.int64)
    dst = np.asarray(edge_list[1], np.int64)

    percore = []
    max_lo = 1
    max_hi = 1
    for c in range(N_CORES):
        base = c * npc
        sel = (src >= base) & (src < base + npc)
        sl = (src[sel] - base).astype(np.int32)
        dg = dst[sel].astype(np.int32)
        order = np.argsort(sl, kind="stable")
        sl, dg = sl[order], dg[order]
        dl = (dg - base) % n
        win = sl >> 7
        bounds = np.searchsorted(win, np.arange(nwin + 1))
        wins = []
        for w in range(nwin):
            s, e = bounds[w], bounds[w + 1]
            srel_w = (sl[s:e] - w * P).astype(np.int32)
            entries = {}
            for layer, d in ((1, dl[s:e]), (2, dg[s:e])):
                lo = d < half
                n_lo = int(lo.sum())
                n_hi = int((~lo).sum())
                max_lo = max(max_lo, math.ceil(n_lo / P))
                max_hi = max(max_hi, math.ceil(n_hi / P))
                entries[layer] = (srel_w[lo], d[lo], srel_w[~lo], d[~lo] - half)
            wins.append(entries)
        percore.append(wins)

    T_LO, T_HI = max_lo, max_hi
    T = T_LO + T_HI
    streams = []
    for c in range(N_CORES):
        out = {}
        for layer in (1, 2):
            ilo = np.zeros((nwin, P, T_LO * 8), np.int16)
            ihi = np.zeros((nwin, P, T_HI * 8), np.int16)
            isrc = np.zeros((nwin, P, T * 8), np.int16)
            srel = np.full((nwin, P, T), -1.0, np.float32)
            for w in range(nwin):
                sr_lo, d_lo, sr_hi, d_hi = percore[c][w][layer]
                lo_idx = np.zeros(T_LO * P, np.int32)
                lo_idx[:d_lo.shape[0]] = d_lo
                hi_idx = np.zeros(T_HI * P, np.int32)
                hi_idx[:d_hi.shape[0]] = d_hi
                src_idx = np.zeros(T * P, np.int32)
                sr_all = np.full(T * P, -1.0, np.float32)
                # slot order: [lo slots | hi slots]
                sr_all[:sr_lo.shape[0]] = sr_lo
                sr_all[T_LO * P:T_LO * P + sr_hi.shape[0]] = sr_hi
                # src (owned, local) index stream for the a_src gather
                w0 = w * P
                src_idx[:sr_lo.shape[0]] = sr_lo + w0
                src_idx[T_LO * P:T_LO * P + sr_hi.shape[0]] = sr_hi + w0
                ilo[w] = _pack_idx(lo_idx)
                ihi[w] = _pack_idx(hi_idx)
                isrc[w] = _pack_idx(src_idx)
                srel[w] = _pack_srel(sr_all)
            out[f"ilo{layer}"] = ilo
            out[f"ihi{layer}"] = ihi
            out[f"isrc{layer}"] = isrc
            out[f"srel{layer}"] = srel
        streams.append(out)
    return streams, T_LO, T_HI


# ---------------------------------------------------------------- program

def _build_program(cfg, T_LO, T_HI, phases="ABCDE", repeats=1):
    import concourse.bacc as bacc
    import concourse.mybir as mybir
    import concourse.tile as tile
    from concourse.masks import make_identity

    n, npc, half, nwin = cfg["n"], cfg["npc"], cfg["half"], cfg["nwin"]
    T = T_LO + T_HI
    f32 = mybir.dt.float32
    i16 = mybir.dt.int16
    ntile_h = math.ceil(n / P)

    skip = set(os.environ.get("GAT_SKIP", "").split(","))
    nc = bacc.Bacc("TRN2", target_bir_lowering=False, debug=False,
                   num_devices=N_CORES)

    xT = nc.dram_tensor("xT", [NFEAT, n], f32, kind="ExternalInput")
    w1aug = nc.dram_tensor("w1aug", [NFEAT, R1], f32, kind="ExternalInput")
    w2aug = nc.dram_tensor("w2aug", [2, NFEAT, R2], f32, kind="ExternalInput")
    b1b = nc.dram_tensor("b1b", [P, F1], f32, kind="ExternalInput")
    b2m = nc.dram_tensor("b2m", [P, D2], f32, kind="ExternalInput")
    iota_in = nc.dram_tensor("iota", [P, P], f32, kind="ExternalInput")
    estream = {}
    for layer in (1, 2):
        estream[f"ilo{layer}"] = nc.dram_tensor(f"ilo{layer}", [nwin, P, T_LO * 8], i16, kind="ExternalInput")
        estream[f"ihi{layer}"] = nc.dram_tensor(f"ihi{layer}", [nwin, P, T_HI * 8], i16, kind="ExternalInput")
        estream[f"isrc{layer}"] = nc.dram_tensor(f"isrc{layer}", [nwin, P, T * 8], i16, kind="ExternalInput")
        estream[f"srel{layer}"] = nc.dram_tensor(f"srel{layer}", [nwin, P, T], f32, kind="ExternalInput")
    out = nc.dram_tensor("out", [npc, D2], f32, kind="ExternalOutput")

    table1 = nc.dram_tensor("table1", [n, ROW1], f32, kind="Internal")
    h2loc = nc.dram_tensor("h2loc", [npc, F1], f32, kind="Internal")
    g2loc = nc.dram_tensor("g2loc", [npc, ROW2], f32, kind="Internal")
    shared = os.environ.get("GAT_SHARED", "0") == "1"
    table2 = nc.dram_tensor("table2", [n, ROW2], f32, kind="Internal",
                            addr_space="Shared" if shared else "Local")

    with tile.TileContext(nc) as tc:
      with (
            tc.tile_pool(name="const", bufs=1) as cpool,
            tc.tile_pool(name="hphase", bufs=3) as hpool,
            tc.tile_pool(name="hpsum", bufs=2, space="PSUM") as hpsum,
            tc.tile_pool(name="edge", bufs=2) as epool,
            tc.tile_pool(name="epsum", bufs=2, space="PSUM") as epsum,
            tc.tile_pool(name="epi", bufs=2) as ipool,
      ):
        for _rep in range(repeats):
            # ---- constants
            w1sb = cpool.tile([NFEAT, R1], f32)
            nc.sync.dma_start(w1sb[:], w1aug[:])
            w2sb = cpool.tile([NFEAT, 2 * R2], f32)
            nc.sync.dma_start(w2sb[:, 0:R2], w2aug[0])
            nc.sync.dma_start(w2sb[:, R2:2 * R2], w2aug[1])
            b1sb = cpool.tile([P, F1], f32)
            nc.sync.dma_start(b1sb[:], b1b[:])
            b2sb = cpool.tile([P, D2], f32)
            nc.sync.dma_start(b2sb[:], b2m[:])
            iotasb = cpool.tile([P, P], f32)
            nc.sync.dma_start(iotasb[:], iota_in[:])
            idsb = cpool.tile([P, P], f32)
            make_identity(nc, idsb[:])

            # ---- phase A: table1[i] = [a_src | a_dst | h] for all n nodes
            CH = 512
            for i in range(math.ceil(n / CH) if "A" in phases else 0):
                n0 = i * CH
                m = min(CH, n - n0)
                nsub = math.ceil(m / P)
                nfull = m // P
                xt = hpool.tile([NFEAT, CH], f32, tag="xt")
                nc.sync.dma_start(xt[:, :m], xT[:, n0:n0 + m])
                sbA = hpool.tile([P, CH // P, R1], f32, tag="sbA")
                for j in range(nsub):
                    nn = min(P, m - j * P)
                    psA = hpsum.tile([P, R1], f32, tag="psA", bufs=4)
                    nc.tensor.matmul(psA[:nn, :], lhsT=xt[:, j * P:j * P + nn],
                                     rhs=w1sb[:], start=True, stop=True)
                    nc.scalar.copy(sbA[:nn, j, :], psA[:nn, :])
                if nfull:
                    dst = table1[n0:n0 + nfull * P, 0:R1].rearrange(
                        "(j p) c -> p j c", p=P)
                    nc.sync.dma_start(dst, sbA[:, 0:nfull, :])
                if nfull < nsub:
                    nn = m - nfull * P
                    nc.sync.dma_start(
                        table1[n0 + nfull * P:n0 + m, 0:R1],
                        sbA[:nn, nfull, :])

            # ---- edge phase (shared for both layers)
            def edge_layer(layer, table, atable, AROW, H, F, AB, ROW, R, epilogue):
                ilo_t, ihi_t = estream[f"ilo{layer}"], estream[f"ihi{layer}"]
                isrc_t, srel_t = estream[f"isrc{layer}"], estream[f"srel{layer}"]
                for w in range(nwin):
                    NW = min(P, npc - w * P)
                    ilo = epool.tile([P, T_LO * 8], i16, tag="ilo")
                    nc.sync.dma_start(ilo[:], ilo_t[w])
                    ihi = epool.tile([P, T_HI * 8], i16, tag="ihi")
                    nc.sync.dma_start(ihi[:], ihi_t[w])
                    isc = epool.tile([P, T * 8], i16, tag="isc")
                    nc.sync.dma_start(isc[:], isrc_t[w])
                    srl = epool.tile([P, T], f32, tag="srl")
                    nc.sync.dma_start(srl[:], srel_t[w])

                    D = epool.tile([P, T, ROW], f32, tag="D")
                    A = epool.tile([P, T, 64], f32, tag="A")
                    if "gd" not in skip:
                        nc.gpsimd.dma_gather(
                            out_ap=D[:, 0:T_LO, 0:ROW], in_ap=table[0:half, :],
                            idxs_ap=ilo[:], num_idxs=T_LO * P,
                            num_idxs_reg=T_LO * P, elem_size=ROW,
                            single_packet=False)
                        nc.gpsimd.dma_gather(
                            out_ap=D[:, T_LO:T, 0:ROW], in_ap=table[half:, :],
                            idxs_ap=ihi[:], num_idxs=T_HI * P,
                            num_idxs_reg=T_HI * P, elem_size=ROW,
                            single_packet=False)
                    if "ga" not in skip:
                        nc.gpsimd.dma_gather(
                            out_ap=A[:, :, :], in_ap=atable[:, 0:64],
                            idxs_ap=isc[:], num_idxs=T * P,
                            num_idxs_reg=T * P, elem_size=64, elem_step=AROW,
                            single_packet=False)

                    # e = leaky_relu(a_src[src] + a_dst[dst]); p = exp(e)
                    et = epool.tile([P, T, H1], f32, tag="et")
                    nc.vector.tensor_tensor(
                        out=et[:, :, 0:H], in0=A[:, :, 0:H],
                        in1=D[:, :, H:2 * H], op=mybir.AluOpType.add)
                    e2 = epool.tile([P, T, H1], f32, tag="e2")
                    nc.vector.tensor_scalar_mul(e2[:, :, 0:H], et[:, :, 0:H], ALPHA)
                    nc.vector.tensor_tensor(
                        out=et[:, :, 0:H], in0=et[:, :, 0:H], in1=e2[:, :, 0:H],
                        op=mybir.AluOpType.max)
                    nc.scalar.activation(D[:, :, 0:H], et[:, :, 0:H],
                                         mybir.ActivationFunctionType.Exp)
                    # scale features by p (per-head broadcast)
                    if "pm" not in skip:
                        feat = D[:, :, AB:AB + F].rearrange(
                            "p t (h d) -> p t h d", h=H)
                        pb = D[:, :, 0:H].unsqueeze(3).broadcast_to(
                            [P, T, H, F // H])
                        nc.vector.tensor_tensor(out=feat, in0=feat, in1=pb,
                                                op=mybir.AluOpType.mult)
                    # one-hot S
                    S = epool.tile([P, T, P], f32, tag="S")
                    if "sb" not in skip:
                        nc.vector.tensor_tensor(
                            out=S[:],
                            in0=iotasb[:].unsqueeze(1).broadcast_to([P, T, P]),
                            in1=srl[:].unsqueeze(2).broadcast_to([P, T, P]),
                            op=mybir.AluOpType.is_equal)
                    ps = epsum.tile([P, R], f32, tag="psW")
                    if "mm" not in skip:
                        for t in range(T):
                            nc.tensor.matmul(ps[:], lhsT=S[:, t, :],
                                             rhs=D[:, t, 0:R],
                                             start=(t == 0), stop=(t == T - 1))
                    epilogue(ps, w, NW)

            # ---- layer-1 epilogue: h2 = elu(hp/denom + b1) -> h2loc
            def epi1(ps, w, NW):
                dn = ipool.tile([P, H1], f32, tag="dn")
                nc.vector.tensor_scalar_add(dn[:, 0:H1], ps[:, 0:H1], 1e-30)
                rr = ipool.tile([P, H1], f32, tag="rr")
                nc.vector.reciprocal(rr[:, 0:H1], dn[:, 0:H1])
                hp = ipool.tile([P, F1], f32, tag="hp")
                nc.vector.tensor_tensor(
                    out=hp[:].rearrange("p (h d) -> p h d", h=H1),
                    in0=ps[:, AB1:R1].rearrange("p (h d) -> p h d", h=H1),
                    in1=rr[:, 0:H1].unsqueeze(2).broadcast_to([P, H1, D1]),
                    op=mybir.AluOpType.mult)
                nc.vector.tensor_tensor(out=hp[:], in0=hp[:], in1=b1sb[:],
                                        op=mybir.AluOpType.add)
                # elu = max(x,0) + (exp(min(x,0)) - 1)
                mn = ipool.tile([P, F1], f32, tag="mn")
                nc.vector.tensor_scalar_min(mn[:], hp[:], 0.0)
                ex = ipool.tile([P, F1], f32, tag="ex")
                nc.scalar.activation(ex[:], mn[:],
                                     mybir.ActivationFunctionType.Exp)
                nc.vector.tensor_scalar_add(ex[:], ex[:], -1.0)
                nc.vector.tensor_scalar_max(hp[:], hp[:], 0.0)
                nc.vector.tensor_tensor(out=hp[:], in0=hp[:], in1=ex[:],
                                        op=mybir.AluOpType.add)
                nc.sync.dma_start(h2loc[w * P:w * P + NW, :], hp[:NW, :])

            if "B" in phases:
                edge_layer(1, table1, table1, ROW1, H1, F1, AB1, ROW1, R1, epi1)

            # ---- phase C: g2 = [a2 | h2 @ W2aug] for owned nodes
            for w in range(nwin if "C" in phases else 0):
                n0 = w * P
                NW = min(P, npc - n0)
                h2sb = hpool.tile([P, F1], f32, tag="h2sb")
                nc.sync.dma_start(h2sb[:NW, :], h2loc[n0:n0 + NW, :])
                hT = hpool.tile([P, 2, P], f32, tag="hT")
                for k in range(2):
                    psT = hpsum.tile([P, P], f32, tag="psT", bufs=1)
                    nc.tensor.transpose(psT[:], h2sb[:, k * P:(k + 1) * P],
                                        idsb[:])
                    nc.scalar.copy(hT[:, k, :], psT[:])
                ps2 = hpsum.tile([P, R2], f32, tag="ps2", bufs=1)
                nc.tensor.matmul(ps2[:], lhsT=hT[:, 0, :], rhs=w2sb[:, 0:R2],
                                 start=True, stop=False)
                nc.tensor.matmul(ps2[:], lhsT=hT[:, 1, :],
                                 rhs=w2sb[:, R2:2 * R2], start=False, stop=True)
                g2sb = hpool.tile([P, R2], f32, tag="g2sb")
                nc.scalar.copy(g2sb[:], ps2[:])
                nc.sync.dma_start(g2loc[n0:n0 + NW, 0:R2], g2sb[:NW, :])

            # ---- phase D: share g2 across cores
            if "D" in phases:
                nc.gpsimd.collective_compute(
                    "AllGather", mybir.AluOpType.bypass,
                    replica_groups=[list(range(N_CORES))],
                    ins=[g2loc[:].opt()], outs=[table2[:].opt()])

            # ---- layer-2 epilogue: log_softmax(mean of heads + b2) -> out
            def epi2(ps, w, NW):
                dn = ipool.tile([P, H2], f32, tag="dn2")
                nc.vector.tensor_scalar_add(dn[:, 0:H2], ps[:, 0:H2], 1e-30)
                rr = ipool.tile([P, H2], f32, tag="rr2")
                nc.vector.reciprocal(rr[:, 0:H2], dn[:, 0:H2])
                nc.vector.tensor_scalar_mul(rr[:, 0:H2], rr[:, 0:H2], 0.5)
                o = ipool.tile([P, D2], f32, tag="o")
                t1 = ipool.tile([P, D2], f32, tag="t1")
                nc.vector.tensor_scalar(
                    out=o[:], in0=ps[:, AB2:AB2 + D2], scalar1=rr[:, 0:1],
                    scalar2=None, op0=mybir.AluOpType.mult)
                nc.vector.tensor_scalar(
                    out=t1[:], in0=ps[:, AB2 + D2:AB2 + 2 * D2],
                    scalar1=rr[:, 1:2], scalar2=None, op0=mybir.AluOpType.mult)
                nc.vector.tensor_tensor(out=o[:], in0=o[:], in1=t1[:],
                                        op=mybir.AluOpType.add)
                nc.vector.tensor_tensor(out=o[:], in0=o[:], in1=b2sb[:],
                                        op=mybir.AluOpType.add)
                nmx = ipool.tile([P, 1], f32, tag="nmx")
                nc.vector.tensor_reduce(out=nmx[:], in_=o[:],
                                        axis=mybir.AxisListType.X,
                                        op=mybir.AluOpType.max, negate=True)
                exs = ipool.tile([P, D2], f32, tag="exs")
                sm = ipool.tile([P, 1], f32, tag="sm")
                nc.scalar.activation(exs[:], o[:],
                                     mybir.ActivationFunctionType.Exp,
                                     bias=nmx[:, 0:1], accum_out=sm[:, 0:1])
                lg = ipool.tile([P, 1], f32, tag="lg")
                nc.scalar.activation(lg[:, 0:1], sm[:, 0:1],
                                     mybir.ActivationFunctionType.Ln)
                res = ipool.tile([P, D2], f32, tag="res")
                nc.vector.tensor_scalar(
                    out=res[:], in0=o[:], scalar1=nmx[:, 0:1],
                    scalar2=lg[:, 0:1], op0=mybir.AluOpType.add,
                    op1=mybir.AluOpType.subtract)
                nc.sync.dma_start(out[w * P:w * P + NW, :], res[:NW, :])

            if "E" in phases:
                edge_layer(2, table2, g2loc, ROW2, H2, F2, AB2, ROW2, R2, epi2)

    nc.compile()
    return nc


def _host_inputs(x, edge_list, W1, a1, b1, W2, a2, b2, cfg):
    w1aug, w2aug, b1b, b2m = _prep_weights(W1, a1, b1, W2, a2, b2)
    streams, T_LO, T_HI = _prep_edges(edge_list, cfg)
    iota = np.tile(np.arange(P, dtype=np.float32).reshape(1, P), (P, 1)).copy()
    x = np.asarray(x, np.float32)
    in_maps = []
    for c in range(N_CORES):
        base = c * cfg["npc"]
        xrot = np.roll(x, -base, axis=0)
        m = dict(
            xT=np.ascontiguousarray(xrot.T),
            w1aug=w1aug, w2aug=w2aug, b1b=b1b, b2m=b2m, iota=iota,
        )
        m.update(streams[c])
        in_maps.append(m)
    return in_maps, T_LO, T_HI


def kernel(x, edge_list, W1, a1, b1, W2, a2, b2):
    from concourse.bass_utils import run_bass_kernel_spmd

    cfg = _cfg_full()
    in_maps, T_LO, T_HI = _host_inputs(x, edge_list, W1, a1, b1, W2, a2, b2, cfg)
    nc = _build_program(cfg, T_LO, T_HI)
    res = run_bass_kernel_spmd(nc, in_maps, core_ids=list(range(N_CORES)))
    return np.concatenate([res.results[c]["out"] for c in range(N_CORES)],
                          axis=0)
